# revision 23
# baseline (speedup 1.0000x reference)
"""Distributed multi-head attention for Trainium2 (8 NeuronCores).

Problem: B=4, S=2048, D=1024, 16 heads x 64 dim, fp32 I/O.
  q/k/v = hs @ W{q,k,v}.T ; scores = (q/8) @ k.T per (b,h);
  attn = softmax(scores) @ v ; out = attn @ Wo.T

Sharding (tensor-parallel over heads + all-to-all):
  - Each core owns 2 heads (128 channels of Wq/Wk/Wv rows).
  - Every core receives the full hidden_states; computes qT/kT/vT for its
    2 heads over all B*S rows; attention in transposed (scoresT) layout so
    softmax sums come free via a ones-augmented v (no max subtraction:
    scores ~ N(0,1)); per-q normalization deferred off the critical path.
  - AllToAll redistributes attn_T (bf16): shard j = this core's 2 heads
    for global row block j. After A2A each core holds all 1024 channels
    for its 1024 rows and applies the full Wo locally -> output row shard.

Schedule: batch-0 hsT via PE transposes (PE idle in prologue); later
batches via bf16 DRAM round-trip + DMA-transpose, prefetched during the
previous batch's attention; QKV chains of batch b+1 interleaved between
attention units of batch b so the in-order PE never bulk-stalls the exp.
"""
import numpy as np

B, S, D = 4, 2048, 1024
NCORE = 8
HD = 64
HPC = 2
CPC = HPC * HD               # 128
ROWS = B * S
RPC = ROWS // NCORE          # 1024

_CACHE = {}


def _build():
    import concourse.bass as bass
    import concourse.bacc as bacc
    import concourse.mybir as mybir
    import concourse.tile as tile
    from concourse.masks import make_identity

    F32 = mybir.dt.float32
    BF16 = mybir.dt.bfloat16
    AF = mybir.ActivationFunctionType

    nc = bacc.Bacc("TRN2", target_bir_lowering=False, debug=False,
                   num_devices=NCORE)
    hs = nc.dram_tensor("hidden_states", [B, S, D], F32, kind="ExternalInput")
    wq = nc.dram_tensor("Wq", [CPC, D], F32, kind="ExternalInput")
    wk = nc.dram_tensor("Wk", [CPC, D], F32, kind="ExternalInput")
    wv = nc.dram_tensor("Wv", [CPC, D], F32, kind="ExternalInput")
    wo = nc.dram_tensor("Wo", [D, D], F32, kind="ExternalInput")
    out = nc.dram_tensor("out", [RPC, D], F32, kind="ExternalOutput")
    bounce_in = nc.dram_tensor("bounce_in", [4, NCORE, CPC, RPC // 4], BF16)
    bounce_out = nc.dram_tensor("bounce_out", [4, NCORE, CPC, RPC // 4], BF16)
    hs16 = nc.dram_tensor("hs16", [B, S, D], BF16)
    cc_warm_in = nc.dram_tensor("cc_warm_in", [NCORE, 128], BF16)
    cc_warm_out = nc.dram_tensor("cc_warm_out", [NCORE, 128], BF16)

    hs_t = [hs[b].rearrange("(t p) d -> p t d", p=128) for b in range(B)]
    hs16_t = [hs16[b].rearrange("(t p) d -> p t d", p=128) for b in range(B)]

    with tile.TileContext(nc) as tc:
        with (
            tc.tile_pool(name="const", bufs=1) as cpool,
            tc.tile_pool(name="persist", bufs=1) as pp,
            tc.tile_pool(name="hsT", bufs=2) as hsT_pool,
            tc.tile_pool(name="proj", bufs=2) as proj_pool,
            tc.tile_pool(name="hload", bufs=3) as hload,
            tc.tile_pool(name="wload", bufs=3) as wload,
            tc.tile_pool(name="sb", bufs=2) as sb,
            tc.tile_pool(name="ex", bufs=3) as expool,
            tc.tile_pool(name="ps_sc", bufs=2, space="PSUM") as ps_sc,
            tc.tile_pool(name="ps_av", bufs=2, space="PSUM") as ps_av,
            tc.tile_pool(name="ps_m", bufs=2, space="PSUM") as ps_m,
        ):
            ident = cpool.tile([128, 128], BF16, tag="ident")
            make_identity(nc, ident)

            # tiny early A2A: absorbs collective setup + rank sync so the
            # real all-to-alls at the tail start hot
            warm = sb.tile([NCORE, 128], BF16, tag="warm", name="warm")
            nc.gpsimd.memset(warm, 0.0)
            nc.gpsimd.dma_start(cc_warm_in[:, :], warm)
            nc.gpsimd.collective_compute(
                "AllToAll", mybir.AluOpType.bypass,
                replica_groups=[list(range(NCORE))],
                ins=[cc_warm_in[:]], outs=[cc_warm_out[:]])

            # ---------- builders ----------
            def hs_pe_pipeline(b):
                """hsT via PE transposes (used for batch 0: PE is idle)."""
                hsT = hsT_pool.tile([128, 8, S], BF16, tag="hsT",
                                    name="hsT")
                for rt in range(16):
                    hf = hload.tile([128, 1, D], F32, tag="hf", name="hf")
                    nc.gpsimd.dma_start(hf, hs_t[b][:, rt:rt + 1, :])
                    hb = hload.tile([128, 1, D], BF16, tag="hb", name="hb")
                    nc.vector.tensor_copy(hb, hf)
                    tp = ps_m.tile([128, 8, 128], BF16, tag="m", name="tp")
                    for kc in range(8):
                        nc.tensor.transpose(
                            tp[:, kc, :], hb[:, 0, kc * 128:(kc + 1) * 128],
                            ident)
                    nc.vector.tensor_copy(
                        hsT.rearrange("p c (t r) -> p c t r", r=128)
                        [:, :, rt, :], tp)
                return hsT

            def hs_dma_pipeline(b):
                """hsT via bf16 DRAM round-trip + DMA transpose (b >= 1)."""
                for g in range(16):
                    hf = hload.tile([128, 1, D], F32, tag="hf", name="hf")
                    nc.gpsimd.dma_start(hf, hs_t[b][:, g:g + 1, :])
                    hb = hload.tile([128, 1, D], BF16, tag="hb", name="hb")
                    nc.vector.tensor_copy(hb, hf)
                    nc.gpsimd.dma_start(hs16_t[b][:, g:g + 1, :], hb)
                hsT = hsT_pool.tile([128, 8, S], BF16, tag="hsT",
                                    name="hsT")
                for kc in range(8):
                    nc.sync.dma_start_transpose(
                        hsT[:, kc, :], hs16[b, :, kc * 128:(kc + 1) * 128])
                return hsT

            def alloc_proj():
                qT = proj_pool.tile([128, S], BF16, tag="qT", name="qT")
                kTt = proj_pool.tile([128, S], BF16, tag="kT", name="kT")
                vTt = proj_pool.tile([128, S], BF16, tag="vT", name="vT")
                vaug = proj_pool.tile([128, HPC, 16, 65], BF16, tag="vaug",
                                      name="vaug")
                return {"q": qT, "k": kTt, "v": vTt, "vaug": vaug}

            def emit_qkv_chain(hsT, prj, p, rb):
                wt = wT[p]
                pq = ps_m.tile([128, 512], F32, tag="m", name="pq")
                for kc in range(8):
                    nc.tensor.matmul(
                        pq, wt[:, kc, :],
                        hsT[:, kc, rb * 512:(rb + 1) * 512],
                        start=(kc == 0), stop=(kc == 7))
                dslice = prj[p][:, rb * 512:(rb + 1) * 512]
                if p == "q":
                    nc.vector.tensor_scalar_mul(dslice, pq, 0.125)
                else:
                    nc.vector.tensor_copy(dslice, pq)

            def emit_vaug(prj, h):
                vTt, vaug = prj["v"], prj["vaug"]
                idh = ident[h * 64:(h + 1) * 64, h * 64:(h + 1) * 64]
                for rt in range(16):
                    pt = ps_m.tile([128, 64], BF16, tag="m", name="pt")
                    nc.tensor.transpose(
                        pt, vTt[h * 64:(h + 1) * 64,
                                rt * 128:(rt + 1) * 128], idh)
                    nc.vector.tensor_copy(vaug[:, h, rt, 0:64], pt)
                    nc.vector.memset(vaug[:, h, rt, 64:65], 1.0)

            def emit_attention_unit(b, prj, h, qp):
                """One (head, q-1024) attention unit; AV lags by one kp."""
                qT, kTt, vaug = prj["q"], prj["k"], prj["vaug"]
                hsl = slice(h * 64, (h + 1) * 64)
                q0 = qp * 1024
                av0 = ps_av.tile([128, 512], F32, tag="av", name="av0")
                av1 = ps_av.tile([128, 512], F32, tag="av", name="av1")
                exs = {}
                for kp in range(16):
                    sc = ps_sc.tile([128, 1024], F32, tag="sc", name="sc")
                    lk = kTt[hsl, kp * 128:(kp + 1) * 128]
                    nc.tensor.matmul(sc[:, 0:512], lk,
                                     qT[hsl, q0:q0 + 512],
                                     start=True, stop=True)
                    nc.tensor.matmul(sc[:, 512:1024], lk,
                                     qT[hsl, q0 + 512:q0 + 1024],
                                     start=True, stop=True)
                    ex = expool.tile([128, 1024], BF16, tag="ex", name="ex")
                    nc.scalar.activation(ex, sc, AF.Exp)
                    exs[kp] = ex
                    if kp >= 1:
                        pex = exs.pop(kp - 1)
                        va = vaug[:, h, kp - 1, :]
                        nc.tensor.matmul(av0[0:65, :], va, pex[:, 0:512],
                                         start=(kp == 1), stop=False)
                        nc.tensor.matmul(av1[0:65, :], va, pex[:, 512:1024],
                                         start=(kp == 1), stop=False)
                pex = exs.pop(15)
                va = vaug[:, h, 15, :]
                nc.tensor.matmul(av0[0:65, :], va, pex[:, 0:512],
                                 start=False, stop=True)
                nc.tensor.matmul(av1[0:65, :], va, pex[:, 512:1024],
                                 start=False, stop=True)
                j = b * 2 + qp
                for half, av in ((0, av0), (1, av1)):
                    ssum = sb.tile([1, 512], F32, tag="ssum", name="ssum")
                    nc.vector.tensor_copy(ssum, av[64:65, :])
                    avf = sb.tile([64, 512], F32, tag="avf", name="avf")
                    nc.vector.tensor_copy(avf, av[0:64, :])
                    recip = sb.tile([1, 512], F32, tag="recip", name="recip")
                    nc.vector.reciprocal_approx_fast(recip, ssum)
                    bc = sb.tile([64, 512], F32, tag="bc", name="bc")
                    nc.gpsimd.partition_broadcast(bc, recip)
                    at = sb.tile([64, 512], BF16, tag="at", name="at")
                    nc.vector.tensor_mul(at, avf, bc)
                    nc.sync.dma_start(
                        bounce_in[2 * half, j, hsl, :], at[:, 0:256])
                    nc.sync.dma_start(
                        bounce_in[2 * half + 1, j, hsl, :], at[:, 256:512])

            # ---------- prologue: weights (q/k/v), then batch 0 ----------
            wT = {}
            for pname, w in (("q", wq), ("k", wk), ("v", wv)):
                wf = wload.tile([128, D], F32, tag="wf", name="wf")
                nc.sync.dma_start(wf, w[:, :])
                wb = wload.tile([128, D], BF16, tag="wb", name="wb")
                nc.vector.tensor_copy(wb, wf)
                wtp = ps_m.tile([128, 8, 128], BF16, tag="m", name="wtp")
                for kc in range(8):
                    nc.tensor.transpose(
                        wtp[:, kc, :], wb[:, kc * 128:(kc + 1) * 128], ident)
                wt = pp.tile([128, 8, 128], BF16, tag=f"wT{pname}",
                             name=f"wT{pname}")
                nc.vector.tensor_copy(wt, wtp)
                wT[pname] = wt

            # batch-0 hsT via PE transposes, qkv chains interleaved per
            # 4-rowtile group so attention can start ~50us in
            hsT_cur = hsT_pool.tile([128, 8, S], BF16, tag="hsT",
                                    name="hsT")
            prj_cur = alloc_proj()
            hsT0v = hsT_cur.rearrange("p c (t r) -> p c t r", r=128)
            for grp in range(4):
                for rt in range(grp * 4, grp * 4 + 4):
                    hf = hload.tile([128, 1, D], F32, tag="hf", name="hf")
                    nc.gpsimd.dma_start(hf, hs_t[0][:, rt:rt + 1, :])
                    hb = hload.tile([128, 1, D], BF16, tag="hb", name="hb")
                    nc.vector.tensor_copy(hb, hf)
                    tp = ps_m.tile([128, 8, 128], BF16, tag="m", name="tp")
                    for kc in range(8):
                        nc.tensor.transpose(
                            tp[:, kc, :], hb[:, 0, kc * 128:(kc + 1) * 128],
                            ident)
                    nc.vector.tensor_copy(hsT0v[:, :, rt, :], tp)
                for p, rb in (("v", grp), ("k", grp)) +                         ((("q", grp),) if grp < 2 else ()):
                    emit_qkv_chain(hsT_cur, prj_cur, p, rb)
            for h in range(HPC):
                emit_vaug(prj_cur, h)

            woT = [pp.tile([128, D], BF16, tag=f"woT{i}", name=f"woT{i}")
                   for i in range(8)]

            # ---------- main loop ----------
            for b in range(B):
                if b + 1 < B:
                    hsT_next = hs_dma_pipeline(b + 1)
                    prj_next = alloc_proj()
                units = [(h, qp) for h in range(HPC) for qp in range(2)]
                for u, (h, qp) in enumerate(units):
                    emit_attention_unit(b, prj_cur, h, qp)
                    if u == 0:
                        emit_qkv_chain(hsT_cur, prj_cur, "q", 2)
                        emit_qkv_chain(hsT_cur, prj_cur, "q", 3)
                    if b + 1 < B and u == 1:
                        for p, rb in (("v", 0), ("v", 1), ("v", 2), ("v", 3),
                                      ("k", 0), ("k", 1)):
                            emit_qkv_chain(hsT_next, prj_next, p, rb)
                    if b + 1 < B and u == 2:
                        for p, rb in (("k", 2), ("k", 3),
                                      ("q", 0), ("q", 1)):
                            emit_qkv_chain(hsT_next, prj_next, p, rb)
                        emit_vaug(prj_next, 0)
                        emit_vaug(prj_next, 1)
                if b + 1 < B:
                    hsT_cur, prj_cur = hsT_next, prj_next

            # ---------- all-to-all (four row-quarter collectives) ----------
            for quart in range(4):
                nc.gpsimd.collective_compute(
                    "AllToAll", mybir.AluOpType.bypass,
                    replica_groups=[list(range(NCORE))],
                    ins=[bounce_in[quart]], outs=[bounce_out[quart]])

            # woT prep here: fills PE idle while the A2As run
            for j in range(8):
                wf = wload.tile([128, D], F32, tag="wf", name="wf")
                nc.sync.dma_start(wf, wo[j * 128:(j + 1) * 128, :])
                wb = wload.tile([128, D], BF16, tag="wb", name="wb")
                nc.vector.tensor_copy(wb, wf)
                wtp = ps_m.tile([128, 8, 128], BF16, tag="m", name="wtp")
                for i in range(8):
                    nc.tensor.transpose(
                        wtp[:, i, :], wb[:, i * 128:(i + 1) * 128], ident)
                for i in range(8):
                    nc.vector.tensor_copy(
                        woT[i][:, j * 128:(j + 1) * 128], wtp[:, i, :])

            # ---------- output projection, per row-quarter ----------
            for quart in range(4):
                rcv = []
                for i in range(8):
                    rc = pp.tile([128, RPC // 4], BF16,
                                 tag=f"rcv{quart}_{i}", name=f"rcv{quart}_{i}")
                    nc.gpsimd.dma_start(rc, bounce_out[quart, i])
                    rcv.append(rc)
                for mm_ in range(2):
                    m = quart * 2 + mm_
                    for chalf in range(2):
                        po = ps_sc.tile([128, 512], F32, tag="sc", name="po")
                        for i in range(8):
                            nc.tensor.matmul(
                                po, rcv[i][:, mm_ * 128:(mm_ + 1) * 128],
                                woT[i][:, chalf * 512:(chalf + 1) * 512],
                                start=(i == 0), stop=(i == 7))
                        osb = sb.tile([128, 512], F32, tag="osb", name="osb")
                        nc.vector.tensor_copy(osb, po)
                        nc.sync.dma_start(
                            out[m * 128:(m + 1) * 128,
                                chalf * 512:(chalf + 1) * 512], osb)

    nc.compile()
    return nc


def _get_nc():
    if "nc" not in _CACHE:
        _CACHE["nc"] = _build()
    return _CACHE["nc"]


def kernel(hidden_states, Wq, Wk, Wv, Wo):
    from concourse.bass_utils import run_bass_kernel_spmd

    hidden_states = np.ascontiguousarray(hidden_states, dtype=np.float32)
    Wq = np.ascontiguousarray(Wq, dtype=np.float32)
    Wk = np.ascontiguousarray(Wk, dtype=np.float32)
    Wv = np.ascontiguousarray(Wv, dtype=np.float32)
    Wo = np.ascontiguousarray(Wo, dtype=np.float32)

    nc = _get_nc()
    in_maps = []
    for c in range(NCORE):
        sl = slice(c * CPC, (c + 1) * CPC)
        in_maps.append({
            "hidden_states": hidden_states,
            "Wq": np.ascontiguousarray(Wq[sl]),
            "Wk": np.ascontiguousarray(Wk[sl]),
            "Wv": np.ascontiguousarray(Wv[sl]),
            "Wo": Wo,
        })
    res = run_bass_kernel_spmd(nc, in_maps, list(range(NCORE)))
    full = np.concatenate([res.results[c]["out"] for c in range(NCORE)],
                          axis=0)
    return full.reshape(B, S, D).astype(np.float32)


# revision 24
# speedup vs baseline: 1.1770x; 1.1770x over previous
"""Distributed multi-head attention for Trainium2 (8 NeuronCores).

Problem: B=4, S=2048, D=1024, 16 heads x 64 dim, fp32 I/O.
  q/k/v = hs @ W{q,k,v}.T ; scores = (q/8) @ k.T per (b,h);
  attn = softmax(scores) @ v ; out = attn @ Wo.T

Sharding (tensor-parallel over heads + all-to-all):
  - Each core owns 2 heads (128 channels of Wq/Wk/Wv rows).
  - Every core receives the full hidden_states; computes qT/kT/vT for its
    2 heads over all B*S rows; attention in transposed (scoresT) layout so
    softmax sums come free via a ones-augmented v (no max subtraction:
    scores ~ N(0,1)); per-q normalization deferred off the critical path.
  - AllToAll redistributes attn_T (bf16): shard j = this core's 2 heads
    for global row block j. After A2A each core holds all 1024 channels
    for its 1024 rows and applies the full Wo locally -> output row shard.

Schedule: batch-0 hsT via PE transposes (PE idle in prologue); later
batches via bf16 DRAM round-trip + DMA-transpose, prefetched during the
previous batch's attention; QKV chains of batch b+1 interleaved between
attention units of batch b so the in-order PE never bulk-stalls the exp.
"""
import numpy as np

B, S, D = 4, 2048, 1024
NCORE = 8
HD = 64
HPC = 2
CPC = HPC * HD               # 128
ROWS = B * S
RPC = ROWS // NCORE          # 1024

_CACHE = {}


def _build():
    import concourse.bass as bass
    import concourse.bacc as bacc
    import concourse.mybir as mybir
    import concourse.tile as tile
    from concourse.masks import make_identity

    F32 = mybir.dt.float32
    BF16 = mybir.dt.bfloat16
    AF = mybir.ActivationFunctionType

    nc = bacc.Bacc("TRN2", target_bir_lowering=False, debug=False,
                   num_devices=NCORE)
    hs = nc.dram_tensor("hidden_states", [B, S, D], F32, kind="ExternalInput")
    wq = nc.dram_tensor("Wq", [CPC, D], F32, kind="ExternalInput")
    wk = nc.dram_tensor("Wk", [CPC, D], F32, kind="ExternalInput")
    wv = nc.dram_tensor("Wv", [CPC, D], F32, kind="ExternalInput")
    wo = nc.dram_tensor("Wo", [D, D], F32, kind="ExternalInput")
    out = nc.dram_tensor("out", [RPC, D], F32, kind="ExternalOutput")
    bounce_in = nc.dram_tensor("bounce_in", [4, NCORE, CPC, RPC // 4], BF16)
    bounce_out = nc.dram_tensor("bounce_out", [4, NCORE, CPC, RPC // 4], BF16)
    hs16 = nc.dram_tensor("hs16", [B, S, D], BF16)
    cc_warm_in = nc.dram_tensor("cc_warm_in", [NCORE, 128], BF16)
    cc_warm_out = nc.dram_tensor("cc_warm_out", [NCORE, 128], BF16)

    hs_t = [hs[b].rearrange("(t p) d -> p t d", p=128) for b in range(B)]
    hs16_t = [hs16[b].rearrange("(t p) d -> p t d", p=128) for b in range(B)]

    with tile.TileContext(nc) as tc:
        with (
            tc.tile_pool(name="const", bufs=1) as cpool,
            tc.tile_pool(name="persist", bufs=1) as pp,
            tc.tile_pool(name="hsT", bufs=2) as hsT_pool,
            tc.tile_pool(name="proj", bufs=2) as proj_pool,
            tc.tile_pool(name="hload", bufs=3) as hload,
            tc.tile_pool(name="wload", bufs=3) as wload,
            tc.tile_pool(name="sb", bufs=2) as sb,
            tc.tile_pool(name="ex", bufs=3) as expool,
            tc.tile_pool(name="ps_sc", bufs=2, space="PSUM") as ps_sc,
            tc.tile_pool(name="ps_av", bufs=2, space="PSUM") as ps_av,
            tc.tile_pool(name="ps_m", bufs=2, space="PSUM") as ps_m,
        ):
            ident = cpool.tile([128, 128], BF16, tag="ident")
            make_identity(nc, ident)

            # tiny early A2A: absorbs collective setup + rank sync so the
            # real all-to-alls at the tail start hot
            warm = sb.tile([NCORE, 128], BF16, tag="warm", name="warm")
            nc.gpsimd.memset(warm, 0.0)
            nc.gpsimd.dma_start(cc_warm_in[:, :], warm)
            nc.gpsimd.collective_compute(
                "AllToAll", mybir.AluOpType.bypass,
                replica_groups=[list(range(NCORE))],
                ins=[cc_warm_in[:]], outs=[cc_warm_out[:]])

            # ---------- builders ----------
            def hs_pe_pipeline(b):
                """hsT via PE transposes (used for batch 0: PE is idle)."""
                hsT = hsT_pool.tile([128, 8, S], BF16, tag="hsT",
                                    name="hsT")
                for rt in range(16):
                    hf = hload.tile([128, 1, D], F32, tag="hf", name="hf")
                    nc.gpsimd.dma_start(hf, hs_t[b][:, rt:rt + 1, :])
                    hb = hload.tile([128, 1, D], BF16, tag="hb", name="hb")
                    nc.vector.tensor_copy(hb, hf)
                    tp = ps_m.tile([128, 8, 128], BF16, tag="m", name="tp")
                    for kc in range(8):
                        nc.tensor.transpose(
                            tp[:, kc, :], hb[:, 0, kc * 128:(kc + 1) * 128],
                            ident)
                    nc.vector.tensor_copy(
                        hsT.rearrange("p c (t r) -> p c t r", r=128)
                        [:, :, rt, :], tp)
                return hsT

            def hs_dma_pipeline(b):
                """hsT via bf16 DRAM round-trip + DMA transpose (b >= 1)."""
                for g in range(16):
                    hf = hload.tile([128, 1, D], F32, tag="hf", name="hf")
                    nc.gpsimd.dma_start(hf, hs_t[b][:, g:g + 1, :])
                    hb = hload.tile([128, 1, D], BF16, tag="hb", name="hb")
                    nc.vector.tensor_copy(hb, hf)
                    nc.gpsimd.dma_start(hs16_t[b][:, g:g + 1, :], hb)
                hsT = hsT_pool.tile([128, 8, S], BF16, tag="hsT",
                                    name="hsT")
                for kc in range(8):
                    nc.sync.dma_start_transpose(
                        hsT[:, kc, :], hs16[b, :, kc * 128:(kc + 1) * 128])
                return hsT

            def alloc_proj():
                qT = proj_pool.tile([128, S], BF16, tag="qT", name="qT")
                kTt = proj_pool.tile([128, S], BF16, tag="kT", name="kT")
                vTt = proj_pool.tile([128, S], BF16, tag="vT", name="vT")
                vaug = proj_pool.tile([128, HPC, 16, 65], BF16, tag="vaug",
                                      name="vaug")
                return {"q": qT, "k": kTt, "v": vTt, "vaug": vaug}

            def emit_qkv_chain(hsT, prj, p, rb):
                wt = wT[p]
                pq = ps_m.tile([128, 512], F32, tag="m", name="pq")
                for kc in range(8):
                    nc.tensor.matmul(
                        pq, wt[:, kc, :],
                        hsT[:, kc, rb * 512:(rb + 1) * 512],
                        start=(kc == 0), stop=(kc == 7))
                dslice = prj[p][:, rb * 512:(rb + 1) * 512]
                if p == "q":
                    nc.vector.tensor_scalar_mul(dslice, pq, 0.125)
                else:
                    nc.vector.tensor_copy(dslice, pq)

            def emit_vaug(prj, h):
                vTt, vaug = prj["v"], prj["vaug"]
                idh = ident[h * 64:(h + 1) * 64, h * 64:(h + 1) * 64]
                for rt in range(16):
                    pt = ps_m.tile([128, 64], BF16, tag="m", name="pt")
                    nc.tensor.transpose(
                        pt, vTt[h * 64:(h + 1) * 64,
                                rt * 128:(rt + 1) * 128], idh)
                    nc.vector.tensor_copy(vaug[:, h, rt, 0:64], pt)
                    nc.vector.memset(vaug[:, h, rt, 64:65], 1.0)

            def emit_attention_unit(b, prj, qc):
                """One q-512 unit, BOTH heads: scores MMs pair on disjoint
                PE row-halves (h0 rows 0-63, h1 rows 64-127) and overlap;
                one exp covers both heads; AV lags by one kp."""
                qT, kTt, vaug = prj["q"], prj["k"], prj["vaug"]
                q0 = qc * 512
                avs = [ps_av.tile([128, 512], F32, tag="av", name=f"av{h}")
                       for h in range(2)]
                exs = {}
                for kp in range(16):
                    sc = ps_sc.tile([128, 2, 512], F32, tag="sc", name="sc")
                    for h in range(2):
                        hsl = slice(h * 64, (h + 1) * 64)
                        nc.tensor.matmul(
                            sc[:, h, :], kTt[hsl, kp * 128:(kp + 1) * 128],
                            qT[hsl, q0:q0 + 512], start=True, stop=True)
                    ex = expool.tile([128, 2, 512], BF16, tag="ex", name="ex")
                    nc.scalar.activation(ex, sc, AF.Exp)
                    exs[kp] = ex
                    if kp >= 1:
                        pex = exs.pop(kp - 1)
                        for h in range(2):
                            nc.tensor.matmul(
                                avs[h][0:65, :], vaug[:, h, kp - 1, :],
                                pex[:, h, :], start=(kp == 1), stop=False)
                pex = exs.pop(15)
                for h in range(2):
                    nc.tensor.matmul(avs[h][0:65, :], vaug[:, h, 15, :],
                                     pex[:, h, :], start=False, stop=True)
                j = b * 2 + qc // 2
                qh = qc % 2
                for h in range(2):
                    hsl = slice(h * 64, (h + 1) * 64)
                    av = avs[h]
                    ssum = sb.tile([1, 512], F32, tag="ssum", name="ssum")
                    nc.vector.tensor_copy(ssum, av[64:65, :])
                    avf = sb.tile([64, 512], F32, tag="avf", name="avf")
                    nc.vector.tensor_copy(avf, av[0:64, :])
                    recip = sb.tile([1, 512], F32, tag="recip", name="recip")
                    nc.vector.reciprocal_approx_fast(recip, ssum)
                    bc = sb.tile([64, 512], F32, tag="bc", name="bc")
                    nc.gpsimd.partition_broadcast(bc, recip)
                    at = sb.tile([64, 512], BF16, tag="at", name="at")
                    nc.vector.tensor_mul(at, avf, bc)
                    nc.sync.dma_start(
                        bounce_in[2 * qh, j, hsl, :], at[:, 0:256])
                    nc.sync.dma_start(
                        bounce_in[2 * qh + 1, j, hsl, :], at[:, 256:512])

            # ---------- prologue: weights (q/k/v), then batch 0 ----------
            wT = {}
            for pname, w in (("q", wq), ("k", wk), ("v", wv)):
                wf = wload.tile([128, D], F32, tag="wf", name="wf")
                nc.sync.dma_start(wf, w[:, :])
                wb = wload.tile([128, D], BF16, tag="wb", name="wb")
                nc.vector.tensor_copy(wb, wf)
                wtp = ps_m.tile([128, 8, 128], BF16, tag="m", name="wtp")
                for kc in range(8):
                    nc.tensor.transpose(
                        wtp[:, kc, :], wb[:, kc * 128:(kc + 1) * 128], ident)
                wt = pp.tile([128, 8, 128], BF16, tag=f"wT{pname}",
                             name=f"wT{pname}")
                nc.vector.tensor_copy(wt, wtp)
                wT[pname] = wt

            # batch-0 hsT via PE transposes, qkv chains interleaved per
            # 4-rowtile group so attention can start ~50us in
            hsT_cur = hsT_pool.tile([128, 8, S], BF16, tag="hsT",
                                    name="hsT")
            prj_cur = alloc_proj()
            hsT0v = hsT_cur.rearrange("p c (t r) -> p c t r", r=128)
            for grp in range(4):
                for rt in range(grp * 4, grp * 4 + 4):
                    hf = hload.tile([128, 1, D], F32, tag="hf", name="hf")
                    nc.gpsimd.dma_start(hf, hs_t[0][:, rt:rt + 1, :])
                    hb = hload.tile([128, 1, D], BF16, tag="hb", name="hb")
                    nc.vector.tensor_copy(hb, hf)
                    tp = ps_m.tile([128, 8, 128], BF16, tag="m", name="tp")
                    for kc in range(8):
                        nc.tensor.transpose(
                            tp[:, kc, :], hb[:, 0, kc * 128:(kc + 1) * 128],
                            ident)
                    nc.vector.tensor_copy(hsT0v[:, :, rt, :], tp)
                for p, rb in (("v", grp), ("k", grp)) +                         ((("q", grp),) if grp < 2 else ()):
                    emit_qkv_chain(hsT_cur, prj_cur, p, rb)
            for h in range(HPC):
                emit_vaug(prj_cur, h)

            woT = [pp.tile([128, D], BF16, tag=f"woT{i}", name=f"woT{i}")
                   for i in range(8)]

            # ---------- main loop ----------
            for b in range(B):
                if b + 1 < B:
                    hsT_next = hs_dma_pipeline(b + 1)
                    prj_next = alloc_proj()
                for u in range(4):
                    emit_attention_unit(b, prj_cur, u)
                    if u == 0:
                        emit_qkv_chain(hsT_cur, prj_cur, "q", 2)
                    if u == 1:
                        emit_qkv_chain(hsT_cur, prj_cur, "q", 3)
                        if b + 1 < B:
                            for p, rb in (("v", 0), ("v", 1), ("v", 2),
                                          ("v", 3), ("k", 0)):
                                emit_qkv_chain(hsT_next, prj_next, p, rb)
                    if b + 1 < B and u == 2:
                        for p, rb in (("k", 1), ("k", 2), ("k", 3),
                                      ("q", 0)):
                            emit_qkv_chain(hsT_next, prj_next, p, rb)
                        emit_vaug(prj_next, 0)
                    if b + 1 < B and u == 3:
                        emit_qkv_chain(hsT_next, prj_next, "q", 1)
                        emit_vaug(prj_next, 1)
                if b + 1 < B:
                    hsT_cur, prj_cur = hsT_next, prj_next

            # ---------- all-to-all (four row-quarter collectives) ----------
            for quart in range(4):
                nc.gpsimd.collective_compute(
                    "AllToAll", mybir.AluOpType.bypass,
                    replica_groups=[list(range(NCORE))],
                    ins=[bounce_in[quart]], outs=[bounce_out[quart]])

            # woT prep here: fills PE idle while the A2As run
            for j in range(8):
                wf = wload.tile([128, D], F32, tag="wf", name="wf")
                nc.sync.dma_start(wf, wo[j * 128:(j + 1) * 128, :])
                wb = wload.tile([128, D], BF16, tag="wb", name="wb")
                nc.vector.tensor_copy(wb, wf)
                wtp = ps_m.tile([128, 8, 128], BF16, tag="m", name="wtp")
                for i in range(8):
                    nc.tensor.transpose(
                        wtp[:, i, :], wb[:, i * 128:(i + 1) * 128], ident)
                for i in range(8):
                    nc.vector.tensor_copy(
                        woT[i][:, j * 128:(j + 1) * 128], wtp[:, i, :])

            # ---------- output projection, per row-quarter ----------
            for quart in range(4):
                rcv = []
                for i in range(8):
                    rc = pp.tile([128, RPC // 4], BF16,
                                 tag=f"rcv{quart}_{i}", name=f"rcv{quart}_{i}")
                    nc.gpsimd.dma_start(rc, bounce_out[quart, i])
                    rcv.append(rc)
                for mm_ in range(2):
                    m = quart * 2 + mm_
                    for chalf in range(2):
                        po = ps_sc.tile([128, 512], F32, tag="sc", name="po")
                        for i in range(8):
                            nc.tensor.matmul(
                                po, rcv[i][:, mm_ * 128:(mm_ + 1) * 128],
                                woT[i][:, chalf * 512:(chalf + 1) * 512],
                                start=(i == 0), stop=(i == 7))
                        osb = sb.tile([128, 512], F32, tag="osb", name="osb")
                        nc.vector.tensor_copy(osb, po)
                        nc.sync.dma_start(
                            out[m * 128:(m + 1) * 128,
                                chalf * 512:(chalf + 1) * 512], osb)

    nc.compile()
    return nc


def _get_nc():
    if "nc" not in _CACHE:
        _CACHE["nc"] = _build()
    return _CACHE["nc"]


def kernel(hidden_states, Wq, Wk, Wv, Wo):
    from concourse.bass_utils import run_bass_kernel_spmd

    hidden_states = np.ascontiguousarray(hidden_states, dtype=np.float32)
    Wq = np.ascontiguousarray(Wq, dtype=np.float32)
    Wk = np.ascontiguousarray(Wk, dtype=np.float32)
    Wv = np.ascontiguousarray(Wv, dtype=np.float32)
    Wo = np.ascontiguousarray(Wo, dtype=np.float32)

    nc = _get_nc()
    in_maps = []
    for c in range(NCORE):
        sl = slice(c * CPC, (c + 1) * CPC)
        in_maps.append({
            "hidden_states": hidden_states,
            "Wq": np.ascontiguousarray(Wq[sl]),
            "Wk": np.ascontiguousarray(Wk[sl]),
            "Wv": np.ascontiguousarray(Wv[sl]),
            "Wo": Wo,
        })
    res = run_bass_kernel_spmd(nc, in_maps, list(range(NCORE)))
    full = np.concatenate([res.results[c]["out"] for c in range(NCORE)],
                          axis=0)
    return full.reshape(B, S, D).astype(np.float32)
